# revision 1
# baseline (speedup 1.0000x reference)
"""DeepAR (2-layer LSTM, H=512) Trainium2 Bass kernel, 8-core data-parallel.

Model (see reference): x = concat(x_cont, emb0[cat0], emb1[cat1]) [B,T,56]
  -> LSTM(512) -> LSTM(512) -> mu = h@Wmu+bmu ; sigma = softplus(h@Wsig+bsig)

Sharding: batch B=256 split across 8 cores (32 rows each); params replicated.

Per-core device program (matmul operands + elementwise bf16, psum fp32):
  - embeddings: per-128-row-tile indirect DMA gathers (multi-index indirect
    DMA corrupts SBUF on HW) assembled with x_cont + a ones row, then
    PE-transposed into x^T [57, (t,b)] bf16
  - L1 scan, per gate-slice n (n-outer so ACT/DVE overlap later matmuls):
    gates_n [32,512] = [x^T_t;1] @ [Wk1;b1]_n + sum_c h1T_c @ Wr1_c,n
  - h1 PE-transposed each step into h1T history [128, KC, T, 32] bf16
  - xz2 m-tiles (= h1 @ Wk2, full-M matmuls) interleaved into the L1 scan
    4 steps behind the recurrence to fill PE gaps; b2 is folded into the
    L2 input chunk via an [I32;ones] stationary against a double-buffered
    xz station whose row 32 holds b2
  - L2 scan: gates = [I;1] @ [xz2_t;b2] + sum_c h2T_c @ Wr2_c, with the
    output-head slices interleaved every 16 steps
  - head: mu/sigma^T [1, 512] = sum_c WmsT_c @ h2T_hist; mu += bmu (DVE),
    sigma = Ln(Exp(x + bsig) + 1)  (no Softplus ACT table on this build)
"""

import numpy as np
import ml_dtypes

import concourse.bass as bass
import concourse.mybir as mybir
import concourse.tile as tile
from concourse import bacc
from concourse.masks import make_identity

F32 = mybir.dt.float32
BF16 = mybir.dt.bfloat16
I16 = mybir.dt.int16
I32 = mybir.dt.int32

B, T, F = 256, 192, 8
CARD0, CARD1 = 1000, 100
E0, E1 = 32, 16
H = 512
DIN = F + E0 + E1          # 56
G4 = 4 * H                 # 2048
NC_N = 8                   # cores
BSH = B // NC_N            # 32 batch rows per core
R = T * BSH                # 6144 (t,b)-ordered rows per core
KC = H // 128              # 4 recurrent K-chunks
NS = 512                   # matmul free-dim slice
NN = G4 // NS              # 4 N-slices
A = mybir.ActivationFunctionType


def _lstm_scan(nc, tc, pools, layer, xsrc, w_sb, hist, ident_f32, ident_bf16,
               post_step=None):
    """One LSTM layer scan over T steps.

    layer 1: xsrc = xT sbuf tile [64, R] (rows 0..56 = x^T plus ones row),
             w_sb [128, 5, G4] (chunk0 = [Wk1;b1], chunks1-4 = Wr1)
    layer 2: xsrc = (xz2_dram, ld_pool); w_sb [128, 4, G4] = Wr2 chunks
    hist: persistent sbuf tile [128, KC, T, BSH] bf16 written with h^T chunks.
    """
    ew = pools["ew"]
    ps_gates = pools["ps_gates"]
    ps_tr = pools["ps_tr"]

    c_state = pools["state"].tile([BSH, H], BF16)

    for t in range(T):
        # ---- per-gate matmul accumulation: n-outer, chunks inner ----
        # gate n finishes early so ACT/DVE overlap the remaining matmuls
        if layer == 1:
            lhsT0 = xsrc[0:DIN + 1, t * BSH:(t + 1) * BSH]   # [57, 32]
            rhs0 = w_sb[0:DIN + 1, 0, :]                     # [57, G4]
        else:
            # station rows 0..31 <- xz_t (parity double-buffered); row 32 = b2
            xz_dram, station, iones = xsrc
            par = t % 2
            nc.sync.dma_start(out=station[0:BSH, par, :],
                              in_=xz_dram[t * BSH:(t + 1) * BSH, :])
            lhsT0 = iones[0:BSH + 1, 0:BSH]
            rhs0 = station[0:BSH + 1, par, :]
        woff = 1 if layer == 1 else 0
        gate_ps = []
        for n in range(NN):
            g_ps = ps_gates.tile([BSH, NS], F32, tag="g")
            gate_ps.append(g_ps)
            nsl = slice(n * NS, (n + 1) * NS)
            nc.tensor.matmul(g_ps[:], lhsT0, rhs0[:, nsl],
                             start=True, stop=(t == 0))
            if t > 0:
                for c in range(KC):
                    nc.tensor.matmul(g_ps[:], hist[:, c, t - 1, :],
                                     w_sb[:, woff + c, nsl],
                                     start=False, stop=(c == KC - 1))

        # ---- gate nonlinearities (order i, f, g, o), bf16 for DVE 2x ----
        sig_i = ew.tile([BSH, H], BF16)
        sig_f = ew.tile([BSH, H], BF16)
        tan_g = ew.tile([BSH, H], BF16)
        sig_o = ew.tile([BSH, H], BF16)
        nc.scalar.activation(sig_i[:], gate_ps[0][:], A.Sigmoid)
        nc.scalar.activation(sig_f[:], gate_ps[1][:], A.Sigmoid)
        nc.scalar.activation(tan_g[:], gate_ps[2][:], A.Tanh)
        nc.scalar.activation(sig_o[:], gate_ps[3][:], A.Sigmoid)

        # ---- cell/state update ----
        ig = ew.tile([BSH, H], BF16)
        nc.vector.tensor_mul(ig[:], sig_i[:], tan_g[:])
        if t == 0:
            nc.vector.tensor_copy(c_state[:], ig[:])
        else:
            fc = ew.tile([BSH, H], BF16)
            nc.vector.tensor_mul(fc[:], sig_f[:], c_state[:])
            nc.vector.tensor_add(c_state[:], fc[:], ig[:])
        # ---- h^T = sig_o^T * tanh(c^T), all in transposed space:
        # sig_o and c transposes overlap the remaining matmuls; the tail is
        # just ACT tanh on c^T plus one DVE mul that writes hist in place
        ps_so = ps_tr.tile([128, KC * BSH], BF16, tag="so")
        ps_tc = ps_tr.tile([128, KC * BSH], BF16, tag="tc")
        for c in range(KC):
            nc.tensor.transpose(ps_so[:, c * BSH:(c + 1) * BSH],
                                sig_o[:, c * 128:(c + 1) * 128],
                                ident_bf16[0:BSH, 0:BSH])
        soT = ew.tile([128, KC * BSH], BF16)
        nc.vector.tensor_copy(soT[:], ps_so[:])
        for c in range(KC):
            nc.tensor.transpose(ps_tc[:, c * BSH:(c + 1) * BSH],
                                c_state[:, c * 128:(c + 1) * 128],
                                ident_bf16[0:BSH, 0:BSH])
        tan_cT = ew.tile([128, KC * BSH], BF16)
        nc.scalar.activation(tan_cT[:], ps_tc[:], A.Tanh)
        hview = bass.AP(tensor=hist.tensor, offset=hist.offset + t * BSH,
                        ap=[list(hist.ap[0]), [T * BSH, KC], [1, BSH]])
        nc.vector.tensor_mul(hview, soT[:], tan_cT[:])

        if post_step is not None:
            post_step(t)


_NC_CACHE = {}


def build_nc(upto="all"):
    if upto in _NC_CACHE:
        return _NC_CACHE[upto]
    from contextlib import ExitStack
    nc = bacc.Bacc("TRN2", num_devices=NC_N)

    # ---------------- DRAM I/O ----------------
    idx0_d = nc.dram_tensor("idx0", [128, R // 128], I32, kind="ExternalInput")
    idx1_d = nc.dram_tensor("idx1", [128, R // 128], I32, kind="ExternalInput")
    e0t_d = nc.dram_tensor("e0tab", [CARD0, E0], F32, kind="ExternalInput")
    e1t_d = nc.dram_tensor("e1tab", [CARD1, E1], F32, kind="ExternalInput")
    xcr_d = nc.dram_tensor("xcr", [128, R // 128, F], F32, kind="ExternalInput")
    w1_d = nc.dram_tensor("w1", [128, 1 + KC, G4], BF16, kind="ExternalInput")
    wk2_d = nc.dram_tensor("wk2", [128, KC, G4], BF16, kind="ExternalInput")
    w2_d = nc.dram_tensor("w2", [128, KC, G4], BF16, kind="ExternalInput")
    b2_d = nc.dram_tensor("b2v", [1, G4], F32, kind="ExternalInput")
    wms_d = nc.dram_tensor("wms", [128, KC, 2], BF16, kind="ExternalInput")
    bms_d = nc.dram_tensor("bms", [1, 2], F32, kind="ExternalInput")

    mu_d = nc.dram_tensor("mu", [BSH, T], F32, kind="ExternalOutput")
    sg_d = nc.dram_tensor("sigma", [BSH, T], F32, kind="ExternalOutput")
    dbg_d = nc.dram_tensor("dbg", [64, R], F32, kind="ExternalOutput") \
        if upto != "all" else None

    xz2_d = nc.dram_tensor("xz2scratch", [R, G4], BF16)  # internal scratch

    _build_body(nc, upto, locals())
    nc.compile()
    _NC_CACHE[upto] = nc
    return nc


def _build_body(nc, upto, env):
    from contextlib import ExitStack
    idx0_d = env["idx0_d"]; idx1_d = env["idx1_d"]; xcr_d = env["xcr_d"]
    e0t_d = env["e0t_d"]; e1t_d = env["e1t_d"]; w1_d = env["w1_d"]
    MT = R // 128
    wk2_d = env["wk2_d"]; w2_d = env["w2_d"]; b2_d = env["b2_d"]
    wms_d = env["wms_d"]; bms_d = env["bms_d"]; mu_d = env["mu_d"]
    sg_d = env["sg_d"]; xz2_d = env["xz2_d"]; dbg_d = env["dbg_d"]
    with tile.TileContext(nc) as tc, ExitStack() as top:  # noqa: SIM117
        singles = top.enter_context(tc.tile_pool(name="singles", bufs=1))

        # ---------------- constants / weights to SBUF ----------------
        wk2_sb = singles.tile([128, KC, G4], BF16)
        nc.sync.dma_start(out=wk2_sb[:], in_=wk2_d[:])
        w2_sb = singles.tile([128, KC, G4], BF16)
        nc.sync.dma_start(out=w2_sb[:], in_=w2_d[:])
        wms_sb = singles.tile([128, KC, 2], BF16)
        nc.sync.dma_start(out=wms_sb[:], in_=wms_d[:])
        bms_sb = singles.tile([1, 2], F32)
        nc.sync.dma_start(out=bms_sb[:], in_=bms_d[:])

        ident_f32 = singles.tile([128, 128], F32)
        make_identity(nc, ident_f32[:])
        ident_bf16 = singles.tile([128, 128], BF16)
        nc.vector.tensor_copy(ident_bf16[:], ident_f32[:])

        h1T = singles.tile([128, KC, T, BSH], BF16)
        h2T = singles.tile([128, KC, T, BSH], BF16)

        # ------- phases 1+2 share a pool so xT/w1 free before phase 4 -------
        p12 = ExitStack()
        xtp = p12.enter_context(tc.tile_pool(name="xtp", bufs=1))
        w1_sb = xtp.tile([128, 1 + KC, G4], BF16)
        nc.sync.dma_start(out=w1_sb[:], in_=w1_d[:])
        xT = xtp.tile([64, R], BF16)   # rows: 0-31 e0, 32-47 e1, 48-55 xc, 56 ones

        # ---------------- phase 1: build x^T ----------------
        with tc.tile_pool(name="gather", bufs=1) as gp, \
                tc.tile_pool(name="gtr", bufs=2, space="PSUM") as ptr:
            idx0_sb = gp.tile([128, MT], I32)
            nc.sync.dma_start(out=idx0_sb[:], in_=idx0_d[:])
            idx1_sb = gp.tile([128, MT], I32)
            nc.sync.dma_start(out=idx1_sb[:], in_=idx1_d[:])

            # assembled rows: [p, m, 64] = [e0 | e1 | xc | ones(+pad)]
            # NOTE: multi-index indirect DMA is broken on HW (stomps memory);
            # one gather per 128-row tile, single idx column each.
            asm = gp.tile([128, MT, 64], F32)
            nc.vector.memset(asm[:], 1.0)
            xcb = gp.tile([128, MT, F], F32)
            nc.sync.dma_start(out=xcb[:], in_=xcr_d[:])
            nc.vector.tensor_copy(asm[:, :, E0 + E1:DIN], xcb[:])
            for m in range(MT):
                nc.gpsimd.indirect_dma_start(
                    out=asm[:, m, 0:E0], out_offset=None, in_=e0t_d[:],
                    in_offset=bass.IndirectOffsetOnAxis(
                        ap=idx0_sb[:, m:m + 1], axis=0))
                nc.gpsimd.indirect_dma_start(
                    out=asm[:, m, E0:E0 + E1], out_offset=None, in_=e1t_d[:],
                    in_offset=bass.IndirectOffsetOnAxis(
                        ap=idx1_sb[:, m:m + 1], axis=0))
            for m in range(MT):
                ps = ptr.tile([64, 128], F32)
                nc.tensor.transpose(ps[:], asm[:, m, :], ident_f32[:])
                nc.vector.tensor_copy(xT[:, 128 * m:128 * (m + 1)], ps[:])

        if upto == "xT":
            with tc.tile_pool(name="dbgp", bufs=1) as dp:
                dbg_sb = dp.tile([64, R], F32)
                nc.vector.tensor_copy(dbg_sb[:], xT[:])
                nc.sync.dma_start(out=dbg_d[:], in_=dbg_sb[:])
            return
        # ---------------- phase 2: L1 scan + interleaved xz2 m-tiles ----------
        with ExitStack() as ph:
            pools = {
                "ew": ph.enter_context(tc.tile_pool(name="ew1", bufs=2)),
                "state": ph.enter_context(tc.tile_pool(name="st1", bufs=1)),
                "ps_gates": ph.enter_context(
                    tc.tile_pool(name="psg1", bufs=4, space="PSUM")),
                "ps_tr": ph.enter_context(
                    tc.tile_pool(name="pst1", bufs=1, space="PSUM")),
            }
            psxz = ph.enter_context(tc.tile_pool(name="psxz", bufs=2, space="PSUM"))
            xzs = ph.enter_context(tc.tile_pool(name="xzs", bufs=3))

            def xz2_tile(t):
                # after step t = 4m+3, rows for m-tile m are complete
                if (t + 1) % 4 != 0:
                    return
                m = (t + 1) // 4 - 1
                for n in range(NN):
                    nsl = slice(n * NS, (n + 1) * NS)
                    ps = psxz.tile([128, NS], F32, tag="xz")
                    for c in range(KC):
                        lhsT = h1T[:, c, 4 * m:4 * (m + 1), :]
                        nc.tensor.matmul(ps[:], lhsT, wk2_sb[:, c, nsl],
                                         start=(c == 0), stop=(c == KC - 1))
                    xz_sb = xzs.tile([128, NS], BF16, tag="xzs")
                    nc.vector.tensor_copy(xz_sb[:], ps[:])
                    nc.sync.dma_start(
                        out=xz2_d[128 * m:128 * (m + 1), nsl], in_=xz_sb[:])

            _lstm_scan(nc, tc, pools, 1, xT, w1_sb, h1T, ident_f32, ident_bf16,
                       post_step=xz2_tile)
        p12.close()

        # -------- phase 4: L2 scan + interleaved head slices ----------------
        with ExitStack() as ph:
            pools = {
                "ew": ph.enter_context(tc.tile_pool(name="ew2", bufs=2)),
                "state": ph.enter_context(tc.tile_pool(name="st2", bufs=1)),
                "ps_gates": ph.enter_context(
                    tc.tile_pool(name="psg2", bufs=4, space="PSUM")),
                "ps_tr": ph.enter_context(
                    tc.tile_pool(name="pst2", bufs=1, space="PSUM")),
            }
            psh = ph.enter_context(tc.tile_pool(name="psh", bufs=1, space="PSUM"))
            hew = ph.enter_context(tc.tile_pool(name="hew", bufs=2))
            stp = ph.enter_context(tc.tile_pool(name="stp", bufs=1))
            # xz station: rows 0..31 xz (parity-double-buffered), row 32 = b2
            station = stp.tile([64, 2, G4], BF16)
            b2row = bass.AP(tensor=b2_d[:].tensor, offset=0,
                            ap=[[0, 1], [0, 2], [1, G4]])
            nc.gpsimd.dma_start(out=station[BSH:BSH + 1, :, :], in_=b2row)
            # [I32; ones-row] stationary for the xz+b2 chunk
            iones = stp.tile([64, BSH], BF16)
            nc.vector.memset(iones[0:64, :], 0.0)
            nc.vector.tensor_copy(iones[0:BSH, :], ident_bf16[0:BSH, 0:BSH])
            nc.vector.memset(iones[BSH:BSH + 1, :], 1.0)
            TSL = NS // BSH  # 16 timesteps per head slice

            def head_slice(t):
                if (t + 1) % TSL != 0:
                    return
                n = (t + 1) // TSL - 1
                ps_mu = psh.tile([1, NS], F32, tag="hm")
                ps_sg = psh.tile([1, NS], F32, tag="hs")
                for c in range(KC):
                    rhs = h2T[:, c, n * TSL:(n + 1) * TSL, :]
                    nc.tensor.matmul(ps_mu[:], wms_sb[:, c, 0:1], rhs,
                                     start=(c == 0), stop=(c == KC - 1))
                    nc.tensor.matmul(ps_sg[:], wms_sb[:, c, 1:2], rhs,
                                     start=(c == 0), stop=(c == KC - 1))
                mu_sl = hew.tile([1, NS], F32)
                nc.vector.tensor_scalar_add(mu_sl[:], ps_mu[:], bms_sb[0:1, 0:1])
                ex = hew.tile([1, NS], F32)
                sg_sl = hew.tile([1, NS], F32)
                nc.scalar.activation(ex[:], ps_sg[:], A.Exp, bias=bms_sb[0:1, 1:2])
                nc.scalar.activation(sg_sl[:], ex[:], A.Ln, bias=1.0)
                mu_view = bass.AP(tensor=mu_d[:].tensor, offset=n * TSL,
                                  ap=[[0, 1], [1, TSL], [T, BSH]])
                nc.sync.dma_start(out=mu_view, in_=mu_sl[:])
                sg_view = bass.AP(tensor=sg_d[:].tensor, offset=n * TSL,
                                  ap=[[0, 1], [1, TSL], [T, BSH]])
                nc.sync.dma_start(out=sg_view, in_=sg_sl[:])

            _lstm_scan(nc, tc, pools, 2, (xz2_d, station, iones), w2_sb, h2T,
                       ident_f32, ident_bf16, post_step=head_slice)

    return nc


def _marshal(inputs):
    """Host-side shard/layout marshalling (no compute beyond dtype cast/pad)."""
    bf = ml_dtypes.bfloat16
    xc = np.ascontiguousarray(np.asarray(inputs["x_cont"], np.float32))
    cat0 = np.asarray(inputs["cat0"]).astype(np.int32)
    cat1 = np.asarray(inputs["cat1"]).astype(np.int32)
    emb0 = np.asarray(inputs["emb0"], np.float32)
    emb1 = np.asarray(inputs["emb1"], np.float32)
    Wk1 = np.asarray(inputs["Wk1"], np.float32)
    Wr1 = np.asarray(inputs["Wr1"], np.float32)
    b1 = np.asarray(inputs["b1"], np.float32)
    Wk2 = np.asarray(inputs["Wk2"], np.float32)
    Wr2 = np.asarray(inputs["Wr2"], np.float32)
    b2 = np.asarray(inputs["b2"], np.float32)
    Wmu = np.asarray(inputs["Wmu"], np.float32)
    bmu = np.asarray(inputs["bmu"], np.float32)
    Wsig = np.asarray(inputs["Wsig"], np.float32)
    bsig = np.asarray(inputs["bsig"], np.float32)

    e0tab = emb0
    e1tab = emb1

    # xT partition order: 0-31 emb0 dims, 32-47 emb1 dims, 48-55 x_cont, 56 ones
    w1 = np.zeros((128, 1 + KC, G4), bf)
    w1[0:E0, 0, :] = Wk1[F:F + E0, :].astype(bf)
    w1[E0:E0 + E1, 0, :] = Wk1[F + E0:DIN, :].astype(bf)
    w1[E0 + E1:E0 + E1 + F, 0, :] = Wk1[0:F, :].astype(bf)
    w1[DIN, 0, :] = b1.astype(bf)
    for c in range(KC):
        w1[:, 1 + c, :] = Wr1[c * 128:(c + 1) * 128, :].astype(bf)
    wk2 = np.zeros((128, KC, G4), bf)
    w2 = np.zeros((128, KC, G4), bf)
    wms = np.zeros((128, KC, 2), bf)
    for c in range(KC):
        wk2[:, c, :] = Wk2[c * 128:(c + 1) * 128, :].astype(bf)
        w2[:, c, :] = Wr2[c * 128:(c + 1) * 128, :].astype(bf)
        wms[:, c, 0] = Wmu[c * 128:(c + 1) * 128, 0].astype(bf)
        wms[:, c, 1] = Wsig[c * 128:(c + 1) * 128, 0].astype(bf)
    b2v = b2.reshape(1, G4)
    bms = np.array([[float(bmu.reshape(-1)[0]), float(bsig.reshape(-1)[0])]],
                   np.float32)

    MT = R // 128

    def wrap_idx(cat):  # [BSH, T] -> (t,b) rows -> [128, MT] int32
        lin = np.ascontiguousarray(cat.T).reshape(-1)       # (t, b) order
        return np.ascontiguousarray(lin.reshape(MT, 128).T.astype(np.int32))

    in_maps = []
    for cidx in range(NC_N):
        sl = slice(cidx * BSH, (cidx + 1) * BSH)
        xcs = xc[sl]                                        # [32, 192, 8]
        rows = xcs.transpose(1, 0, 2).reshape(R, F)      # (t,b) rows
        xcr = np.ascontiguousarray(
            rows.reshape(MT, 128, F).transpose(1, 0, 2).astype(np.float32))
        in_maps.append({
            "xcr": xcr,
            "idx0": wrap_idx(cat0[sl]),
            "idx1": wrap_idx(cat1[sl]),
            "e0tab": e0tab, "e1tab": e1tab,
            "w1": w1, "wk2": wk2, "w2": w2, "b2v": b2v,
            "wms": wms, "bms": bms,
        })
    return in_maps


_RUN_KWARGS = {}   # test harness may set e.g. {"trace": True} for profiling
_LAST_RESULT = []


def kernel(**inputs):
    from concourse.bass_utils import run_bass_kernel_spmd
    in_maps = _marshal(inputs)
    nc = build_nc()
    res = run_bass_kernel_spmd(nc, in_maps, core_ids=list(range(NC_N)),
                               **_RUN_KWARGS)
    _LAST_RESULT.clear()
    _LAST_RESULT.append(res)
    mu = np.concatenate([r["mu"] for r in res.results], axis=0)      # [256, 192]
    sg = np.concatenate([r["sigma"] for r in res.results], axis=0)
    return (mu.reshape(B, T, 1).astype(np.float32),
            sg.reshape(B, T, 1).astype(np.float32))



# revision 2
# speedup vs baseline: 2.2938x; 2.2938x over previous
"""DeepAR (2-layer LSTM, H=512) Trainium2 Bass kernel, 8-core data-parallel.

Model (see reference): x = concat(x_cont, emb0[cat0], emb1[cat1]) [B,T,56]
  -> LSTM(512) -> LSTM(512) -> mu = h@Wmu+bmu ; sigma = softplus(h@Wsig+bsig)

Sharding: batch B=256 split across 8 cores (32 rows each); params replicated.

Per-core device program (all matmul operands bf16, psum fp32):
  - embeddings: per-128-row-tile indirect DMA gathers (multi-index indirect
    DMA corrupts SBUF on HW) assembled with x_cont + a ones row, then
    PE-transposed into x^T [57, (t,b)] bf16
  - fused transposed-gates scan: gates are computed TRANSPOSED as 16 chunks
    [128 gate dims, 32 batch] with the weight chunk as the PE stationary and
    h^T [128, 32] moving, so each matmul streams only 32 rows (vs 512 when
    batch is the partition dim) and h^T needs no per-step transpose.
    Gate columns are permuted [i, f, o, g] so one sigmoid covers i|f|o.
  - L2 runs one step behind L1 in the same loop (software pipeline): while
    L1 step t's activation tail runs on ACT/DVE, the PE executes L2 step
    t-1 (seed + Wk2(h1) + Wr2(h2)), so the PE never idles on the tail.
    b2 is seeded with a single K=16 one-hot matmul.
  - head: mu/sigma^T [1, 512] = sum_c WmsT_c @ h2T_hist slices every 16
    steps; mu += bmu (DVE), sigma = Ln(Exp(x + bsig) + 1)
"""

import numpy as np
import ml_dtypes

import concourse.bass as bass
import concourse.mybir as mybir
import concourse.tile as tile
from concourse import bacc
from concourse.masks import make_identity

F32 = mybir.dt.float32
BF16 = mybir.dt.bfloat16
I32 = mybir.dt.int32

B, T, F = 256, 192, 8
CARD0, CARD1 = 1000, 100
E0, E1 = 32, 16
H = 512
DIN = F + E0 + E1          # 56
G4 = 4 * H                 # 2048
NC_N = 8                   # cores
BSH = B // NC_N            # 32 batch rows per core
R = T * BSH                # 6144 (t,b)-ordered rows per core
KC = H // 128              # 4 recurrent K-chunks
NM = G4 // 128             # 16 gate-dim chunks
A = mybir.ActivationFunctionType

# chunk emission order: g chunks first (tanh(g) can start early), then i, f, o
_M_ORDER = [12, 13, 14, 15, 0, 1, 2, 3, 4, 5, 6, 7, 8, 9, 10, 11]

_NC_CACHE = {}


def build_nc(upto="all"):
    if upto in _NC_CACHE:
        return _NC_CACHE[upto]
    nc = bacc.Bacc("TRN2", num_devices=NC_N)

    # ---------------- DRAM I/O ----------------
    idx0_d = nc.dram_tensor("idx0", [128, R // 128], I32, kind="ExternalInput")
    idx1_d = nc.dram_tensor("idx1", [128, R // 128], I32, kind="ExternalInput")
    e0t_d = nc.dram_tensor("e0tab", [CARD0, E0], F32, kind="ExternalInput")
    e1t_d = nc.dram_tensor("e1tab", [CARD1, E1], F32, kind="ExternalInput")
    xcr_d = nc.dram_tensor("xcr", [128, R // 128, F], F32, kind="ExternalInput")
    w1x_d = nc.dram_tensor("w1x", [64, G4], BF16, kind="ExternalInput")
    wr1_d = nc.dram_tensor("wr1", [128, KC, G4], BF16, kind="ExternalInput")
    wk2_d = nc.dram_tensor("wk2", [128, KC, G4], BF16, kind="ExternalInput")
    wr2_d = nc.dram_tensor("wr2", [128, KC, G4], BF16, kind="ExternalInput")
    b2t_d = nc.dram_tensor("b2t", [16, 128], BF16, kind="ExternalInput")
    oneh_d = nc.dram_tensor("oneh", [16, 512], BF16, kind="ExternalInput")
    wms_d = nc.dram_tensor("wms", [128, KC, 2], BF16, kind="ExternalInput")
    bms_d = nc.dram_tensor("bms", [1, 2], F32, kind="ExternalInput")

    mu_d = nc.dram_tensor("mu", [BSH, T], F32, kind="ExternalOutput")
    sg_d = nc.dram_tensor("sigma", [BSH, T], F32, kind="ExternalOutput")
    dbg_d = nc.dram_tensor("dbg", [64, R], F32, kind="ExternalOutput") \
        if upto != "all" else None

    _build_body(nc, upto, locals())
    nc.compile()
    _NC_CACHE[upto] = nc
    return nc


def _build_body(nc, upto, env):
    from contextlib import ExitStack
    idx0_d = env["idx0_d"]; idx1_d = env["idx1_d"]; xcr_d = env["xcr_d"]
    e0t_d = env["e0t_d"]; e1t_d = env["e1t_d"]
    w1x_d = env["w1x_d"]; wr1_d = env["wr1_d"]
    wk2_d = env["wk2_d"]; wr2_d = env["wr2_d"]
    b2t_d = env["b2t_d"]; oneh_d = env["oneh_d"]
    wms_d = env["wms_d"]; bms_d = env["bms_d"]; mu_d = env["mu_d"]
    sg_d = env["sg_d"]; dbg_d = env["dbg_d"]
    MT = R // 128
    TSL = 512 // BSH  # 16 timesteps per head slice

    with tile.TileContext(nc) as tc, ExitStack() as top:  # noqa: SIM117
        singles = top.enter_context(tc.tile_pool(name="singles", bufs=1))

        # ---------------- constants / weights to SBUF ----------------
        w1x_sb = singles.tile([64, G4], BF16)
        nc.sync.dma_start(out=w1x_sb[:], in_=w1x_d[:])
        wr1_sb = singles.tile([128, KC, G4], BF16)
        nc.sync.dma_start(out=wr1_sb[:], in_=wr1_d[:])
        wk2_sb = singles.tile([128, KC, G4], BF16)
        nc.sync.dma_start(out=wk2_sb[:], in_=wk2_d[:])
        wr2_sb = singles.tile([128, KC, G4], BF16)
        nc.sync.dma_start(out=wr2_sb[:], in_=wr2_d[:])
        b2t_sb = singles.tile([16, 128], BF16)
        nc.sync.dma_start(out=b2t_sb[:], in_=b2t_d[:])
        oneh_sb = singles.tile([16, 512], BF16)
        nc.sync.dma_start(out=oneh_sb[:], in_=oneh_d[:])
        wms_sb = singles.tile([128, KC, 2], BF16)
        nc.sync.dma_start(out=wms_sb[:], in_=wms_d[:])
        bms_sb = singles.tile([1, 2], F32)
        nc.sync.dma_start(out=bms_sb[:], in_=bms_d[:])

        ident_f32 = singles.tile([128, 128], F32)
        make_identity(nc, ident_f32[:])

        xT = singles.tile([64, R], BF16)  # rows: 0-31 e0, 32-47 e1, 48-55 xc, 56 ones
        h1T = singles.tile([128, 2, KC, BSH], BF16)   # parity double-buffered
        h2T = singles.tile([128, KC, T, BSH], BF16)   # full history (head)
        c1 = singles.tile([128, 128], BF16)
        c2 = singles.tile([128, 128], BF16)
        nc.vector.memset(c1[:], 0.0)
        nc.vector.memset(c2[:], 0.0)

        # ---------------- phase 1: build x^T ----------------
        with tc.tile_pool(name="gather", bufs=1) as gp, \
                tc.tile_pool(name="gtr", bufs=2, space="PSUM") as ptr:
            idx0_sb = gp.tile([128, MT], I32)
            nc.sync.dma_start(out=idx0_sb[:], in_=idx0_d[:])
            idx1_sb = gp.tile([128, MT], I32)
            nc.sync.dma_start(out=idx1_sb[:], in_=idx1_d[:])

            # assembled rows: [p, m, 64] = [e0 | e1 | xc | ones(+pad)]
            # NOTE: multi-index indirect DMA is broken on HW (stomps memory);
            # one gather per 128-row tile, single idx column each.
            asm = gp.tile([128, MT, 64], F32)
            nc.vector.memset(asm[:], 1.0)
            xcb = gp.tile([128, MT, F], F32)
            nc.sync.dma_start(out=xcb[:], in_=xcr_d[:])
            nc.vector.tensor_copy(asm[:, :, E0 + E1:DIN], xcb[:])
            for m in range(MT):
                nc.gpsimd.indirect_dma_start(
                    out=asm[:, m, 0:E0], out_offset=None, in_=e0t_d[:],
                    in_offset=bass.IndirectOffsetOnAxis(
                        ap=idx0_sb[:, m:m + 1], axis=0))
                nc.gpsimd.indirect_dma_start(
                    out=asm[:, m, E0:E0 + E1], out_offset=None, in_=e1t_d[:],
                    in_offset=bass.IndirectOffsetOnAxis(
                        ap=idx1_sb[:, m:m + 1], axis=0))
            for m in range(MT):
                ps = ptr.tile([64, 128], F32)
                nc.tensor.transpose(ps[:], asm[:, m, :], ident_f32[:])
                nc.vector.tensor_copy(xT[:, 128 * m:128 * (m + 1)], ps[:])

        if upto == "xT":
            with tc.tile_pool(name="dbgp", bufs=1) as dp:
                dbg_sb = dp.tile([64, R], F32)
                nc.vector.tensor_copy(dbg_sb[:], xT[:])
                nc.sync.dma_start(out=dbg_d[:], in_=dbg_sb[:])
            return

        # -------- phase 2: fused transposed-gates scan (L2 one step behind) ----
        with ExitStack() as ph:
            ew = ph.enter_context(tc.tile_pool(name="ew", bufs=2))
            pg1p = ph.enter_context(tc.tile_pool(name="pg1", bufs=2, space="PSUM"))
            pg2p = ph.enter_context(tc.tile_pool(name="pg2", bufs=2, space="PSUM"))
            psh = ph.enter_context(tc.tile_pool(name="psh", bufs=1, space="PSUM"))
            hew = ph.enter_context(tc.tile_pool(name="hew", bufs=2))

            def gate_tail(pg, c_st, hview):
                """sigmoid(i|f|o), tanh(g), c' = f*c + i*g, h = o*tanh(c')."""
                tg = ew.tile([128, 128], BF16, tag="tg")
                nc.scalar.activation(tg[:], pg[:, 384:512], A.Tanh)
                sif = ew.tile([128, 384], BF16, tag="sif")
                nc.scalar.activation(sif[:], pg[:, 0:384], A.Sigmoid)
                ig = ew.tile([128, 128], BF16, tag="ig")
                nc.vector.tensor_mul(ig[:], sif[:, 0:128], tg[:])
                fc = ew.tile([128, 128], BF16, tag="fc")
                nc.vector.tensor_mul(fc[:], sif[:, 128:256], c_st[:])
                nc.vector.tensor_add(c_st[:], fc[:], ig[:])
                tc_ = ew.tile([128, 128], BF16, tag="tc")
                nc.scalar.activation(tc_[:], c_st[:], A.Tanh)
                nc.vector.tensor_mul(hview, sif[:, 256:384], tc_[:])

            def l1_step(t):
                par = t % 2
                pg = pg1p.tile([128, 512], F32, tag="pg1")
                for m in _M_ORDER:
                    osl = pg[:, 32 * m:32 * m + 32]
                    nc.tensor.matmul(osl, w1x_sb[0:DIN + 1, 128 * m:128 * (m + 1)],
                                     xT[0:DIN + 1, t * BSH:(t + 1) * BSH],
                                     start=True, stop=(t == 0))
                    if t > 0:
                        for c in range(KC):
                            nc.tensor.matmul(
                                osl, wr1_sb[:, c, 128 * m:128 * (m + 1)],
                                h1T[:, 1 - par, c, :],
                                start=False, stop=(c == KC - 1))
                hv = h1T[:, par, :, :]
                gate_tail(pg, c1, hv)

            def l2_step(s):
                par = s % 2
                pg = pg2p.tile([128, 512], F32, tag="pg2")
                # b2 seed: one K=16 one-hot matmul fills all 16 chunks
                nc.tensor.matmul(pg[:, 0:512], b2t_sb[:], oneh_sb[:],
                                 start=True, stop=False, skip_group_check=True)
                for m in _M_ORDER:
                    osl = pg[:, 32 * m:32 * m + 32]
                    if s > 0:
                        for c in range(KC):
                            nc.tensor.matmul(
                                osl, wr2_sb[:, c, 128 * m:128 * (m + 1)],
                                h2T[:, c, s - 1, :],
                                start=False, stop=False, skip_group_check=True)
                    for c in range(KC):
                        nc.tensor.matmul(
                            osl, wk2_sb[:, c, 128 * m:128 * (m + 1)],
                            h1T[:, par, c, :],
                            start=False, stop=(c == KC - 1),
                            skip_group_check=True)
                hv = bass.AP(tensor=h2T.tensor, offset=h2T.offset + s * BSH,
                             ap=[list(h2T.ap[0]), [T * BSH, KC], [1, BSH]])
                gate_tail(pg, c2, hv)

            def head_slice(n):
                ps_mu = psh.tile([1, 512], F32, tag="hm")
                ps_sg = psh.tile([1, 512], F32, tag="hs")
                for c in range(KC):
                    rhs = h2T[:, c, n * TSL:(n + 1) * TSL, :]
                    nc.tensor.matmul(ps_mu[:], wms_sb[:, c, 0:1], rhs,
                                     start=(c == 0), stop=(c == KC - 1))
                    nc.tensor.matmul(ps_sg[:], wms_sb[:, c, 1:2], rhs,
                                     start=(c == 0), stop=(c == KC - 1))
                mu_sl = hew.tile([1, 512], F32)
                nc.vector.tensor_scalar_add(mu_sl[:], ps_mu[:], bms_sb[0:1, 0:1])
                ex = hew.tile([1, 512], F32)
                sg_sl = hew.tile([1, 512], F32)
                nc.scalar.activation(ex[:], ps_sg[:], A.Exp, bias=bms_sb[0:1, 1:2])
                nc.scalar.activation(sg_sl[:], ex[:], A.Ln, bias=1.0)
                mu_view = bass.AP(tensor=mu_d[:].tensor, offset=n * TSL,
                                  ap=[[0, 1], [1, TSL], [T, BSH]])
                nc.sync.dma_start(out=mu_view, in_=mu_sl[:])
                sg_view = bass.AP(tensor=sg_d[:].tensor, offset=n * TSL,
                                  ap=[[0, 1], [1, TSL], [T, BSH]])
                nc.sync.dma_start(out=sg_view, in_=sg_sl[:])

            for t in range(T + 1):
                if t < T:
                    l1_step(t)
                if t >= 1:
                    s = t - 1
                    l2_step(s)
                    if (s + 1) % TSL == 0:
                        head_slice((s + 1) // TSL - 1)

    return nc


def _marshal(inputs):
    """Host-side shard/layout marshalling (no compute beyond dtype cast/pad)."""
    bf = ml_dtypes.bfloat16
    xc = np.ascontiguousarray(np.asarray(inputs["x_cont"], np.float32))
    cat0 = np.asarray(inputs["cat0"]).astype(np.int32)
    cat1 = np.asarray(inputs["cat1"]).astype(np.int32)
    emb0 = np.asarray(inputs["emb0"], np.float32)
    emb1 = np.asarray(inputs["emb1"], np.float32)
    Wk1 = np.asarray(inputs["Wk1"], np.float32)
    Wr1 = np.asarray(inputs["Wr1"], np.float32)
    b1 = np.asarray(inputs["b1"], np.float32)
    Wk2 = np.asarray(inputs["Wk2"], np.float32)
    Wr2 = np.asarray(inputs["Wr2"], np.float32)
    b2 = np.asarray(inputs["b2"], np.float32)
    Wmu = np.asarray(inputs["Wmu"], np.float32)
    bmu = np.asarray(inputs["bmu"], np.float32)
    Wsig = np.asarray(inputs["Wsig"], np.float32)
    bsig = np.asarray(inputs["bsig"], np.float32)

    # permute gate columns [i, f, g, o] -> [i, f, o, g] so sigmoid gates are
    # contiguous in the transposed-gates free layout
    def perm(W):
        return np.concatenate(
            [W[..., 0:H], W[..., H:2 * H], W[..., 3 * H:4 * H],
             W[..., 2 * H:3 * H]], axis=-1)

    Wk1p, Wr1p, b1p = perm(Wk1), perm(Wr1), perm(b1)
    Wk2p, Wr2p, b2p = perm(Wk2), perm(Wr2), perm(b2)

    # x^T partition order: 0-31 emb0 dims, 32-47 emb1 dims, 48-55 x_cont, 56 ones
    w1x = np.zeros((64, G4), bf)
    w1x[0:E0, :] = Wk1p[F:F + E0, :].astype(bf)
    w1x[E0:E0 + E1, :] = Wk1p[F + E0:DIN, :].astype(bf)
    w1x[E0 + E1:DIN, :] = Wk1p[0:F, :].astype(bf)
    w1x[DIN, :] = b1p.astype(bf)

    wr1 = np.zeros((128, KC, G4), bf)
    wk2 = np.zeros((128, KC, G4), bf)
    wr2 = np.zeros((128, KC, G4), bf)
    wms = np.zeros((128, KC, 2), bf)
    for c in range(KC):
        wr1[:, c, :] = Wr1p[c * 128:(c + 1) * 128, :].astype(bf)
        wk2[:, c, :] = Wk2p[c * 128:(c + 1) * 128, :].astype(bf)
        wr2[:, c, :] = Wr2p[c * 128:(c + 1) * 128, :].astype(bf)
        wms[:, c, 0] = Wmu[c * 128:(c + 1) * 128, 0].astype(bf)
        wms[:, c, 1] = Wsig[c * 128:(c + 1) * 128, 0].astype(bf)
    b2t = np.ascontiguousarray(b2p.reshape(16, 128).astype(bf))
    oneh = np.kron(np.eye(16, dtype=np.float32),
                   np.ones((1, BSH), np.float32)).astype(bf)
    bms = np.array([[float(bmu.reshape(-1)[0]), float(bsig.reshape(-1)[0])]],
                   np.float32)

    MT = R // 128

    def wrap_idx(cat):  # [BSH, T] -> (t,b) rows -> [128, MT] int32
        lin = np.ascontiguousarray(cat.T).reshape(-1)       # (t, b) order
        return np.ascontiguousarray(lin.reshape(MT, 128).T.astype(np.int32))

    in_maps = []
    for cidx in range(NC_N):
        sl = slice(cidx * BSH, (cidx + 1) * BSH)
        xcs = xc[sl]                                        # [32, 192, 8]
        rows = xcs.transpose(1, 0, 2).reshape(R, F)      # (t,b) rows
        xcr = np.ascontiguousarray(
            rows.reshape(MT, 128, F).transpose(1, 0, 2).astype(np.float32))
        in_maps.append({
            "xcr": xcr,
            "idx0": wrap_idx(cat0[sl]),
            "idx1": wrap_idx(cat1[sl]),
            "e0tab": emb0, "e1tab": emb1,
            "w1x": w1x, "wr1": wr1, "wk2": wk2, "wr2": wr2,
            "b2t": b2t, "oneh": oneh,
            "wms": wms, "bms": bms,
        })
    return in_maps


_RUN_KWARGS = {}   # test harness may set e.g. {"trace": True} for profiling
_LAST_RESULT = []


def kernel(**inputs):
    from concourse.bass_utils import run_bass_kernel_spmd
    in_maps = _marshal(inputs)
    nc = build_nc()
    res = run_bass_kernel_spmd(nc, in_maps, core_ids=list(range(NC_N)),
                               **_RUN_KWARGS)
    _LAST_RESULT.clear()
    _LAST_RESULT.append(res)
    mu = np.concatenate([r["mu"] for r in res.results], axis=0)      # [256, 192]
    sg = np.concatenate([r["sigma"] for r in res.results], axis=0)
    return (mu.reshape(B, T, 1).astype(np.float32),
            sg.reshape(B, T, 1).astype(np.float32))


# revision 34
# speedup vs baseline: 3.2989x; 1.4382x over previous
"""DeepAR (2-layer LSTM, H=512) Trainium2 Bass kernel, 8-core data-parallel.

Model (see reference): x = concat(x_cont, emb0[cat0], emb1[cat1]) [B,T,56]
  -> LSTM(512) -> LSTM(512) -> mu = h@Wmu+bmu ; sigma = softplus(h@Wsig+bsig)

Sharding: batch B=256 split across 8 cores (32 rows each); params replicated.

Per-core device program (all matmul operands bf16, psum fp32):
  - embeddings: per-128-row-tile indirect DMA gathers (multi-index indirect
    DMA corrupts SBUF on HW) assembled with x_cont + a ones row, then
    PE-transposed into x^T [57, (t,b)] bf16
  - fused transposed-gates scan: gates are computed TRANSPOSED as 16 chunks
    [128 gate dims, 32 batch] with the weight chunk as the PE stationary and
    h^T [128, 32] moving, so each matmul streams only 32 rows (vs 512 when
    batch is the partition dim) and h^T needs no per-step transpose.
    Gate columns are permuted [i, f, o, g] so one sigmoid covers i|f|o.
  - L2 runs one step behind L1 in the same loop (software pipeline): while
    L1 step t's activation tail runs on ACT/DVE, the PE executes L2 step
    t-1 (seed + Wk2(h1) + Wr2(h2)), so the PE never idles on the tail.
    b2 is seeded with a single K=16 one-hot matmul.
  - head: mu/sigma^T [1, 512] = sum_c WmsT_c @ h2T_hist slices every 16
    steps; mu += bmu (DVE), sigma = Ln(Exp(x + bsig) + 1)
"""

import numpy as np
import ml_dtypes

import concourse.bass as bass
import concourse.mybir as mybir
import concourse.tile as tile
from concourse import bacc
from concourse.masks import make_identity

F32 = mybir.dt.float32
BF16 = mybir.dt.bfloat16
I32 = mybir.dt.int32

B, T, F = 256, 192, 8
CARD0, CARD1 = 1000, 100
E0, E1 = 32, 16
H = 512
DIN = F + E0 + E1          # 56
G4 = 4 * H                 # 2048
NC_N = 8                   # cores
BSH = B // NC_N            # 32 batch rows per core
R = T * BSH                # 6144 (t,b)-ordered rows per core
KC = H // 128              # 4 recurrent K-chunks
NM = G4 // 128             # 16 gate-dim chunks
A = mybir.ActivationFunctionType

# chunk emission order: i,f first (sig(i|f) starts the critical chain),
# then g (tanh(g) ready right before i*g), then o (only needed at the end)
_M_ORDER = [12, 13, 14, 15, 0, 1, 2, 3, 4, 5, 6, 7, 8, 9, 10, 11]
# L2 uses merged sig(i|f|o), so its i,f,o chunks go first (ready mid-run);
# g last feeds only the DVE i*g product, which L2's 2-step slack absorbs
_M_ORDER2 = [0, 1, 2, 3, 4, 5, 6, 7, 8, 9, 10, 11, 12, 13, 14, 15]

_NC_CACHE = {}


def build_nc(upto="all"):
    if upto in _NC_CACHE:
        return _NC_CACHE[upto]
    nc = bacc.Bacc("TRN2", num_devices=NC_N)

    # ---------------- DRAM I/O ----------------
    idx0_d = nc.dram_tensor("idx0", [128, R // 128], I32, kind="ExternalInput")
    idx1r_d = nc.dram_tensor("idx1r", [1, R], BF16, kind="ExternalInput")
    iota1_d = nc.dram_tensor("iota1", [CARD1, 1], F32, kind="ExternalInput")
    e0t_d = nc.dram_tensor("e0tab", [CARD0, E0], F32, kind="ExternalInput")
    e1t_d = nc.dram_tensor("e1t16", [CARD1, E1], BF16, kind="ExternalInput")
    xcr_d = nc.dram_tensor("xcr", [128, R // 128, F], F32, kind="ExternalInput")
    w1x_d = nc.dram_tensor("w1x", [64, G4], BF16, kind="ExternalInput")
    wr1_d = nc.dram_tensor("wr1", [128, KC, G4], BF16, kind="ExternalInput")
    wk2_d = nc.dram_tensor("wk2", [128, KC, G4], BF16, kind="ExternalInput")
    wr2_d = nc.dram_tensor("wr2", [128, KC, G4], BF16, kind="ExternalInput")
    b2t_d = nc.dram_tensor("b2t", [16, 128], BF16, kind="ExternalInput")
    oneh_d = nc.dram_tensor("oneh", [16, 512], BF16, kind="ExternalInput")
    wms_d = nc.dram_tensor("wms", [128, KC, 2], BF16, kind="ExternalInput")
    bms_d = nc.dram_tensor("bms", [1, 2], F32, kind="ExternalInput")

    mu_d = nc.dram_tensor("mu", [BSH, T], F32, kind="ExternalOutput")
    sg_d = nc.dram_tensor("sigma", [BSH, T], F32, kind="ExternalOutput")
    dbg_d = nc.dram_tensor("dbg", [64, R], F32, kind="ExternalOutput") \
        if upto != "all" else None

    _build_body(nc, upto, locals())
    nc.compile()
    _NC_CACHE[upto] = nc
    return nc


def _build_body(nc, upto, env):
    from contextlib import ExitStack
    idx0_d = env["idx0_d"]; idx1r_d = env["idx1r_d"]; xcr_d = env["xcr_d"]
    iota1_d = env["iota1_d"]
    e0t_d = env["e0t_d"]; e1t_d = env["e1t_d"]
    w1x_d = env["w1x_d"]; wr1_d = env["wr1_d"]
    wk2_d = env["wk2_d"]; wr2_d = env["wr2_d"]
    b2t_d = env["b2t_d"]; oneh_d = env["oneh_d"]
    wms_d = env["wms_d"]; bms_d = env["bms_d"]; mu_d = env["mu_d"]
    sg_d = env["sg_d"]; dbg_d = env["dbg_d"]
    MT = R // 128
    TSL = 512 // BSH  # 16 timesteps per head slice

    with tile.TileContext(nc) as tc, ExitStack() as top:  # noqa: SIM117
        singles = top.enter_context(tc.tile_pool(name="singles", bufs=1))
        # scan pools are opened BEFORE the phase-1 pools so they never share
        # PSUM banks / SBUF ranges with them (sharing would add WAR waits
        # serializing scan start behind the last phase-1 gather/transpose)
        ew = top.enter_context(tc.tile_pool(name="ew", bufs=8))
        pg1p = top.enter_context(tc.tile_pool(name="pg1", bufs=1, space="PSUM"))
        pg2p = top.enter_context(tc.tile_pool(name="pg2", bufs=1, space="PSUM"))
        psh = top.enter_context(tc.tile_pool(name="psh", bufs=1, space="PSUM"))
        hew = top.enter_context(tc.tile_pool(name="hew", bufs=2))

        # ---------------- constants / weights to SBUF ----------------
        # small input tensors first so they don't queue behind 6.3MB of
        # weights on the DMA ring (phase 1 needs them immediately)
        singles_idx = singles
        idx0_sb = singles_idx.tile([128, MT], I32)
        nc.sync.dma_start(out=idx0_sb[:], in_=idx0_d[:])
        xcb = singles.tile([128, MT, F], F32)
        nc.sync.dma_start(out=xcb[:], in_=xcr_d[:])
        iota1_sb = singles.tile([CARD1, 1], F32)
        nc.sync.dma_start(out=iota1_sb[:], in_=iota1_d[:])
        e1t_sb = singles.tile([CARD1, E1], BF16)
        nc.sync.dma_start(out=e1t_sb[:], in_=e1t_d[:])
        idx1rep = singles.tile([CARD1, R], BF16)
        idx1_rep_src = bass.AP(tensor=idx1r_d[:].tensor, offset=0,
                               ap=[[0, CARD1], [1, R]])
        nc.sync.dma_start(out=idx1rep[:], in_=idx1_rep_src)
        w1x_sb = singles.tile([64, G4], BF16)
        nc.sync.dma_start(out=w1x_sb[:], in_=w1x_d[:])
        # big weight loads in per-chunk pieces so the short per-tile gather
        # transfers can interleave on the (serial) DMA engine
        wr1_sb = singles.tile([128, KC, G4], BF16)
        for c in range(KC):
            nc.sync.dma_start(out=wr1_sb[:, c, :], in_=wr1_d[:, c, :])
        b2t_sb = singles.tile([16, 128], BF16)
        nc.sync.dma_start(out=b2t_sb[:], in_=b2t_d[:])
        oneh_sb = singles.tile([16, 512], BF16)
        nc.sync.dma_start(out=oneh_sb[:], in_=oneh_d[:])
        wms_sb = singles.tile([128, KC, 2], BF16)
        nc.sync.dma_start(out=wms_sb[:], in_=wms_d[:])
        bms_sb = singles.tile([1, 2], F32)
        nc.sync.dma_start(out=bms_sb[:], in_=bms_d[:])
        wk2_sb = singles.tile([128, KC, G4], BF16)
        for c in range(KC):
            nc.sync.dma_start(out=wk2_sb[:, c, :], in_=wk2_d[:, c, :])
        wr2_sb = singles.tile([128, KC, G4], BF16)
        for c in range(KC):
            nc.sync.dma_start(out=wr2_sb[:, c, :], in_=wr2_d[:, c, :])

        ident_f32 = singles.tile([128, 128], F32)
        make_identity(nc, ident_f32[:])

        # x^T as one tile PER 128-column block: dependencies are tracked at
        # tile granularity, so a single xT tile would make scan step 0 wait
        # for the LAST phase-1 gather (~100us of serial indirect DMAs)
        xTs = [singles.tile([64, 128], BF16, name=f"xT{m}", tag=f"xT{m}")
               for m in range(MT)]
        h1T = singles.tile([128, 4, KC, BSH], BF16)   # 4-deep (L2 lags 2 steps)
        h2T = singles.tile([128, KC, T, BSH], BF16)   # full history (head)
        c1 = singles.tile([128, 128], BF16)
        c2 = singles.tile([128, 128], BF16)
        sgacc = singles.tile([1, T * BSH], F32)   # raw sigma pre-activations
        exp_all = singles.tile([1, T * BSH], BF16)
        nc.vector.memset(c1[:], 0.0)
        nc.vector.memset(c2[:], 0.0)

        # ---------------- phase 1: build x^T ----------------
        # The per-tile work is emitted INTERLEAVED into the scan loop with an
        # 8-tile lookahead: the in-order PE queue otherwise places each
        # transpose far ahead of its gather's completion and every early scan
        # step stalls on the (1us-per-gather) software-DGE pipeline.
        gp = top.enter_context(tc.tile_pool(name="gather", bufs=1))
        ptr = top.enter_context(tc.tile_pool(name="gtr", bufs=1, space="PSUM"))
        # e1 lookup as a one-hot matmul (CARD1=100 <= 128): cheaper than
        # 48 more 1us software-DGE gathers on the Pool engine
        onehot1 = gp.tile([CARD1, R], BF16)
        nc.vector.tensor_scalar(onehot1[:], idx1rep[:], iota1_sb[:],
                                None, op0=mybir.AluOpType.is_equal)

        def emit_tile(m):
            # assembled rows: [p, 64] = [e0 | e1(pad) | xc | ones]
            # NOTE: multi-index indirect DMA is broken on HW (stomps memory);
            # one gather per 128-row tile, single idx column each.
            asm = gp.tile([128, 64], F32, name=f"asm{m}", tag=f"asm{m}")
            nc.vector.memset(asm[:], 1.0)
            nc.gpsimd.tensor_copy(asm[:, E0 + E1:DIN], xcb[:, m, :])
            nc.gpsimd.indirect_dma_start(
                out=asm[:, 0:E0], out_offset=None, in_=e0t_d[:],
                in_offset=bass.IndirectOffsetOnAxis(
                    ap=idx0_sb[:, m:m + 1], axis=0))
            ps = ptr.tile([80, 128], F32, name=f"ps{m}", tag="ps")
            nc.tensor.transpose(ps[0:64, :], asm[:], ident_f32[:])
            nc.vector.tensor_copy(xTs[m][:], ps[0:64, :])
            nc.tensor.matmul(ps[64:80, :], e1t_sb[:],
                             onehot1[:, 128 * m:128 * (m + 1)],
                             start=True, stop=True)
            nc.vector.tensor_copy(xTs[m][E0:E0 + E1, :], ps[64:80, :])

        PRO = 4   # tiles emitted before the scan starts (supply lookahead)

        if upto == "xT":
            for m in range(MT):
                emit_tile(m)
            with tc.tile_pool(name="dbgp", bufs=1) as dp:
                dbg_sb = dp.tile([64, R], F32)
                for m in range(MT):
                    nc.vector.tensor_copy(
                        dbg_sb[:, 128 * m:128 * (m + 1)], xTs[m][:])
                nc.sync.dma_start(out=dbg_d[:], in_=dbg_sb[:])
            return
        for m in range(PRO):
            emit_tile(m)

        # -------- phase 2: fused transposed-gates scan (L2 two steps behind,
        # so every matmul of a block is ready when the block's run starts and
        # L2's psum accumulation group never blocks L1 on the in-order PE
        # queue) --------
        if True:
            # gates psum is split into three tiles per step — (i|f), (o), (g) —
            # because RAW deps are tile-granular: one [128,512] tile would make
            # every gate activation wait for ALL 80 matmuls of the step.
            def gate_slot(pool, tagp, merged):
                if merged:   # (i|f|o) in one tile, g separate
                    return (pool.tile([128, 384], F32, name=tagp + "ifo",
                                      tag=tagp + "ifo"),
                            None,
                            pool.tile([128, 128], F32, name=tagp + "g",
                                      tag=tagp + "g"))
                return (pool.tile([128, 256], F32, name=tagp + "if", tag=tagp + "if"),
                        pool.tile([128, 128], F32, name=tagp + "o", tag=tagp + "o"),
                        pool.tile([128, 128], F32, name=tagp + "g", tag=tagp + "g"))

            def gate_out(pgs, m):
                pg_if, pg_o, pg_g = pgs
                if m >= 12:
                    return pg_g[:, 32 * (m - 12):32 * (m - 12) + 32]
                if pg_o is None:
                    return pg_if[:, 32 * m:32 * m + 32]
                if m < 8:
                    return pg_if[:, 32 * m:32 * m + 32]
                return pg_o[:, 32 * (m - 8):32 * (m - 8) + 32]

            def gate_tail(pgs, c_st, hview, eng):
                """sig(i|f[|o]), tanh(g), c' = f*c + i*g, h = o*tanh(c')."""
                pg_if, pg_o, pg_g = pgs
                tg = ew.tile([128, 128], BF16, tag="tg")
                nc.scalar.activation(tg[:], pg_g[:], A.Tanh)
                nif = 384 if pg_o is None else 256
                sif = ew.tile([128, nif], BF16, tag="sif")
                nc.scalar.activation(sif[:], pg_if[:], A.Sigmoid)
                ig = ew.tile([128, 128], BF16, tag="ig")
                eng.tensor_mul(ig[:], sif[:, 0:128], tg[:])
                fc = ew.tile([128, 128], BF16, tag="fc")
                eng.tensor_mul(fc[:], sif[:, 128:256], c_st[:])
                eng.tensor_add(c_st[:], fc[:], ig[:])
                if pg_o is None:
                    so = sif[:, 256:384]
                else:
                    so_t = ew.tile([128, 128], BF16, tag="so")
                    nc.scalar.activation(so_t[:], pg_o[:], A.Sigmoid)
                    so = so_t[:]
                tc_ = ew.tile([128, 128], BF16, tag="tc")
                nc.scalar.activation(tc_[:], c_st[:], A.Tanh)
                eng.tensor_mul(hview, so, tc_[:])

            def l1_step(t):
                pgs = gate_slot(pg1p, "p1", False)
                xsrc = xTs[t // 4]
                xsl = slice((t % 4) * BSH, (t % 4 + 1) * BSH)
                for m in _M_ORDER:
                    osl = gate_out(pgs, m)
                    nc.tensor.matmul(osl, w1x_sb[0:DIN + 1, 128 * m:128 * (m + 1)],
                                     xsrc[0:DIN + 1, xsl],
                                     start=True, stop=(t == 0))
                    if t > 0:
                        for c in range(KC):
                            nc.tensor.matmul(
                                osl, wr1_sb[:, c, 128 * m:128 * (m + 1)],
                                h1T[:, (t - 1) % 4, c, :],
                                start=False, stop=(c == KC - 1))
                hv = h1T[:, t % 4, :, :]
                gate_tail(pgs, c1, hv, nc.vector)

            def l2_step(s):
                pgs = gate_slot(pg2p, "p2", False)
                # b2 seed: one K=16 one-hot matmul per gate tile
                nc.tensor.matmul(pgs[0][:], b2t_sb[:], oneh_sb[:, 0:256],
                                 start=True, stop=False, skip_group_check=True)
                nc.tensor.matmul(pgs[1][:], b2t_sb[:], oneh_sb[:, 256:384],
                                 start=True, stop=False, skip_group_check=True)
                nc.tensor.matmul(pgs[2][:], b2t_sb[:], oneh_sb[:, 384:512],
                                 start=True, stop=False, skip_group_check=True)
                for m in _M_ORDER:
                    osl = gate_out(pgs, m)
                    if s > 0:
                        for c in range(KC):
                            nc.tensor.matmul(
                                osl, wr2_sb[:, c, 128 * m:128 * (m + 1)],
                                h2T[:, c, s - 1, :],
                                start=False, stop=False, skip_group_check=True)
                    for c in range(KC):
                        nc.tensor.matmul(
                            osl, wk2_sb[:, c, 128 * m:128 * (m + 1)],
                            h1T[:, s % 4, c, :],
                            start=False, stop=(c == KC - 1),
                            skip_group_check=True)
                hv = bass.AP(tensor=h2T.tensor, offset=h2T.offset + s * BSH,
                             ap=[list(h2T.ap[0]), [T * BSH, KC], [1, BSH]])
                gate_tail(pgs, c2, hv, nc.vector)

            def head_slice(n):
                ps_head = psh.tile([33, 512], F32, tag="hm")
                ps_mu = ps_head[0:1, :]
                ps_sg = ps_head[32:33, :]
                for c in range(KC):
                    rhs = h2T[:, c, n * TSL:(n + 1) * TSL, :]
                    nc.tensor.matmul(ps_mu, wms_sb[:, c, 0:1], rhs,
                                     start=(c == 0), stop=(c == KC - 1))
                    nc.tensor.matmul(ps_sg, wms_sb[:, c, 1:2], rhs,
                                     start=(c == 0), stop=(c == KC - 1))
                mu_sl = hew.tile([1, 512], F32)
                nc.vector.tensor_scalar_add(mu_sl[:], ps_mu, bms_sb[0:1, 0:1])
                nc.vector.tensor_copy(sgacc[0:1, 512 * n:512 * (n + 1)], ps_sg)
                mu_view = bass.AP(tensor=mu_d[:].tensor, offset=n * TSL,
                                  ap=[[0, 1], [1, TSL], [T, BSH]])
                nc.sync.dma_start(out=mu_view, in_=mu_sl[:])

            for t in range(T + 2):
                if t % 4 == 0 and PRO + t // 4 < MT:
                    emit_tile(PRO + t // 4)
                if t < T:
                    l1_step(t)
                if t >= 2:
                    s = t - 2
                    l2_step(s)
                    if (s + 1) % TSL == 0:
                        head_slice((s + 1) // TSL - 1)

            # sigma = softplus(raw + bsig), batched once: Exp/Ln tables are in
            # a different act-func set than Sigmoid/Tanh, so doing this inside
            # the scan would cost two 1.3us table swaps every 16 steps
            nc.scalar.activation(exp_all[:], sgacc[:], A.Exp,
                                 bias=bms_sb[0:1, 1:2])
            nc.scalar.activation(sgacc[:], exp_all[:], A.Ln, bias=1.0)
            sg_view = bass.AP(
                tensor=sg_d[:].tensor, offset=0,
                ap=[[0, 1], [TSL, T // TSL], [1, TSL], [T, BSH]])
            nc.sync.dma_start(out=sg_view, in_=sgacc[:])

    return nc


def _marshal(inputs):
    """Host-side shard/layout marshalling (no compute beyond dtype cast/pad)."""
    bf = ml_dtypes.bfloat16
    xc = np.ascontiguousarray(np.asarray(inputs["x_cont"], np.float32))
    cat0 = np.asarray(inputs["cat0"]).astype(np.int32)
    cat1 = np.asarray(inputs["cat1"]).astype(np.int32)
    emb0 = np.asarray(inputs["emb0"], np.float32)
    emb1 = np.asarray(inputs["emb1"], np.float32)
    Wk1 = np.asarray(inputs["Wk1"], np.float32)
    Wr1 = np.asarray(inputs["Wr1"], np.float32)
    b1 = np.asarray(inputs["b1"], np.float32)
    Wk2 = np.asarray(inputs["Wk2"], np.float32)
    Wr2 = np.asarray(inputs["Wr2"], np.float32)
    b2 = np.asarray(inputs["b2"], np.float32)
    Wmu = np.asarray(inputs["Wmu"], np.float32)
    bmu = np.asarray(inputs["bmu"], np.float32)
    Wsig = np.asarray(inputs["Wsig"], np.float32)
    bsig = np.asarray(inputs["bsig"], np.float32)

    # permute gate columns [i, f, g, o] -> [i, f, o, g] so sigmoid gates are
    # contiguous in the transposed-gates free layout
    def perm(W):
        return np.concatenate(
            [W[..., 0:H], W[..., H:2 * H], W[..., 3 * H:4 * H],
             W[..., 2 * H:3 * H]], axis=-1)

    Wk1p, Wr1p, b1p = perm(Wk1), perm(Wr1), perm(b1)
    Wk2p, Wr2p, b2p = perm(Wk2), perm(Wr2), perm(b2)

    # x^T partition order: 0-31 emb0 dims, 32-47 emb1 dims, 48-55 x_cont, 56 ones
    w1x = np.zeros((64, G4), bf)
    w1x[0:E0, :] = Wk1p[F:F + E0, :].astype(bf)
    w1x[E0:E0 + E1, :] = Wk1p[F + E0:DIN, :].astype(bf)
    w1x[E0 + E1:DIN, :] = Wk1p[0:F, :].astype(bf)
    w1x[DIN, :] = b1p.astype(bf)

    wr1 = np.zeros((128, KC, G4), bf)
    wk2 = np.zeros((128, KC, G4), bf)
    wr2 = np.zeros((128, KC, G4), bf)
    wms = np.zeros((128, KC, 2), bf)
    for c in range(KC):
        wr1[:, c, :] = Wr1p[c * 128:(c + 1) * 128, :].astype(bf)
        wk2[:, c, :] = Wk2p[c * 128:(c + 1) * 128, :].astype(bf)
        wr2[:, c, :] = Wr2p[c * 128:(c + 1) * 128, :].astype(bf)
        wms[:, c, 0] = Wmu[c * 128:(c + 1) * 128, 0].astype(bf)
        wms[:, c, 1] = Wsig[c * 128:(c + 1) * 128, 0].astype(bf)
    b2t = np.ascontiguousarray(b2p.reshape(16, 128).astype(bf))
    oneh = np.kron(np.eye(16, dtype=np.float32),
                   np.ones((1, BSH), np.float32)).astype(bf)
    bms = np.array([[float(bmu.reshape(-1)[0]), float(bsig.reshape(-1)[0])]],
                   np.float32)

    MT = R // 128

    def wrap_idx(cat):  # [BSH, T] -> (t,b) rows -> [128, MT] int32
        lin = np.ascontiguousarray(cat.T).reshape(-1)       # (t, b) order
        return np.ascontiguousarray(lin.reshape(MT, 128).T.astype(np.int32))

    iota1 = np.arange(CARD1, dtype=np.float32).reshape(CARD1, 1)
    e1t16 = emb1.astype(bf)

    in_maps = []
    for cidx in range(NC_N):
        sl = slice(cidx * BSH, (cidx + 1) * BSH)
        xcs = xc[sl]                                        # [32, 192, 8]
        rows = xcs.transpose(1, 0, 2).reshape(R, F)      # (t,b) rows
        xcr = np.ascontiguousarray(
            rows.reshape(MT, 128, F).transpose(1, 0, 2).astype(np.float32))
        idx1r = np.ascontiguousarray(
            cat1[sl].T.reshape(1, R).astype(np.float32)).astype(bf)
        in_maps.append({
            "xcr": xcr,
            "idx0": wrap_idx(cat0[sl]),
            "idx1r": idx1r, "iota1": iota1,
            "e0tab": emb0, "e1t16": e1t16,
            "w1x": w1x, "wr1": wr1, "wk2": wk2, "wr2": wr2,
            "b2t": b2t, "oneh": oneh,
            "wms": wms, "bms": bms,
        })
    return in_maps


_RUN_KWARGS = {}   # test harness may set e.g. {"trace": True} for profiling
_LAST_RESULT = []


def kernel(**inputs):
    from concourse.bass_utils import run_bass_kernel_spmd
    in_maps = _marshal(inputs)
    nc = build_nc()
    res = run_bass_kernel_spmd(nc, in_maps, core_ids=list(range(NC_N)),
                               **_RUN_KWARGS)
    _LAST_RESULT.clear()
    _LAST_RESULT.append(res)
    mu = np.concatenate([r["mu"] for r in res.results], axis=0)      # [256, 192]
    sg = np.concatenate([r["sigma"] for r in res.results], axis=0)
    return (mu.reshape(B, T, 1).astype(np.float32),
            sg.reshape(B, T, 1).astype(np.float32))


# revision 37
# speedup vs baseline: 3.3032x; 1.0013x over previous
"""DeepAR (2-layer LSTM, H=512) Trainium2 Bass kernel, 8-core data-parallel.

Model (see reference): x = concat(x_cont, emb0[cat0], emb1[cat1]) [B,T,56]
  -> LSTM(512) -> LSTM(512) -> mu = h@Wmu+bmu ; sigma = softplus(h@Wsig+bsig)

Sharding: batch B=256 split across 8 cores (32 rows each); params replicated.

Per-core device program (all matmul operands bf16, psum fp32):
  - embeddings: e0 via per-128-row-tile indirect DMA gathers (multi-index
    indirect DMA corrupts SBUF on HW; each gather holds the GPSIMD Q7 ~1us),
    e1 via a one-hot matmul (CARD1=100 <= 128): onehot[k,n] = (cat1[n]==k)
    built with one DVE is_equal against a replicated index row. Tiles are
    assembled with x_cont + a ones row and PE-transposed into x^T bf16, one
    SBUF tile per 128 (t,b)-columns (deps are tile-granular), with the
    per-tile work emitted interleaved into the scan loop (lookahead PRO) so
    the in-order PE queue never stalls on a not-yet-gathered tile. Weight
    DMAs are chunked so gather transfers interleave on the serial DMA engine.
  - fused transposed-gates scan: gates are computed TRANSPOSED as 16 chunks
    [128 gate dims, 32 batch] with the weight chunk as the PE stationary and
    h^T [128, 32] moving, so each matmul streams only 32 output rows (vs 512
    with batch on the partition dim) and h^T needs no per-step transpose.
    Gate columns are permuted [i, f, o, g]; the gates psum is split into
    three tiles (i|f, o, g) so each activation waits only its own chunks.
    L2 runs TWO steps behind L1 in the same loop: every matmul of a block is
    ready when the block starts, so the in-order PE queue never waits on the
    activation tail. b2 is seeded with one K=16 one-hot matmul per gate tile.
  - head: mu/sigma^T [1, 128] = sum_c WmsT_c @ h2T_hist every 4 steps;
    mu += bmu (DVE); raw sigma pre-activations are staged in SBUF and
    softplus = Ln(Exp(x + bsig) + 1) runs ONCE batched after the scan
    (Exp/Ln live in a different ACT table set than Sigmoid/Tanh, so inline
    use would cost two 1.3us table swaps every head slice).
"""

import numpy as np
import ml_dtypes

import concourse.bass as bass
import concourse.mybir as mybir
import concourse.tile as tile
from concourse import bacc
from concourse.masks import make_identity

F32 = mybir.dt.float32
BF16 = mybir.dt.bfloat16
I32 = mybir.dt.int32

B, T, F = 256, 192, 8
CARD0, CARD1 = 1000, 100
E0, E1 = 32, 16
H = 512
DIN = F + E0 + E1          # 56
G4 = 4 * H                 # 2048
NC_N = 8                   # cores
BSH = B // NC_N            # 32 batch rows per core
R = T * BSH                # 6144 (t,b)-ordered rows per core
KC = H // 128              # 4 recurrent K-chunks
NM = G4 // 128             # 16 gate-dim chunks
A = mybir.ActivationFunctionType

# chunk emission order: i,f first (sig(i|f) starts the critical chain),
# then g (tanh(g) ready right before i*g), then o (only needed at the end)
_M_ORDER = [12, 13, 14, 15, 0, 1, 2, 3, 4, 5, 6, 7, 8, 9, 10, 11]
# L2 uses merged sig(i|f|o), so its i,f,o chunks go first (ready mid-run);
# g last feeds only the DVE i*g product, which L2's 2-step slack absorbs
_M_ORDER2 = [0, 1, 2, 3, 4, 5, 6, 7, 8, 9, 10, 11, 12, 13, 14, 15]

_NC_CACHE = {}


def build_nc(upto="all"):
    if upto in _NC_CACHE:
        return _NC_CACHE[upto]
    nc = bacc.Bacc("TRN2", num_devices=NC_N)

    # ---------------- DRAM I/O ----------------
    idx0_d = nc.dram_tensor("idx0", [128, R // 128], I32, kind="ExternalInput")
    idx1r_d = nc.dram_tensor("idx1r", [1, R], BF16, kind="ExternalInput")
    iota1_d = nc.dram_tensor("iota1", [CARD1, 1], F32, kind="ExternalInput")
    e0t_d = nc.dram_tensor("e0tab", [CARD0, E0], F32, kind="ExternalInput")
    e1t_d = nc.dram_tensor("e1t16", [CARD1, E1], BF16, kind="ExternalInput")
    xcr_d = nc.dram_tensor("xcr", [128, R // 128, F], F32, kind="ExternalInput")
    w1x_d = nc.dram_tensor("w1x", [64, G4], BF16, kind="ExternalInput")
    wr1_d = nc.dram_tensor("wr1", [128, KC, G4], BF16, kind="ExternalInput")
    wk2_d = nc.dram_tensor("wk2", [128, KC, G4], BF16, kind="ExternalInput")
    wr2_d = nc.dram_tensor("wr2", [128, KC, G4], BF16, kind="ExternalInput")
    b2t_d = nc.dram_tensor("b2t", [16, 128], BF16, kind="ExternalInput")
    oneh_d = nc.dram_tensor("oneh", [16, 512], BF16, kind="ExternalInput")
    wms_d = nc.dram_tensor("wms", [128, KC, 2], BF16, kind="ExternalInput")
    bms_d = nc.dram_tensor("bms", [1, 2], F32, kind="ExternalInput")

    mu_d = nc.dram_tensor("mu", [BSH, T], F32, kind="ExternalOutput")
    sg_d = nc.dram_tensor("sigma", [BSH, T], F32, kind="ExternalOutput")
    dbg_d = nc.dram_tensor("dbg", [64, R], F32, kind="ExternalOutput") \
        if upto != "all" else None

    _build_body(nc, upto, locals())
    nc.compile()
    _NC_CACHE[upto] = nc
    return nc


def _build_body(nc, upto, env):
    from contextlib import ExitStack
    idx0_d = env["idx0_d"]; idx1r_d = env["idx1r_d"]; xcr_d = env["xcr_d"]
    iota1_d = env["iota1_d"]
    e0t_d = env["e0t_d"]; e1t_d = env["e1t_d"]
    w1x_d = env["w1x_d"]; wr1_d = env["wr1_d"]
    wk2_d = env["wk2_d"]; wr2_d = env["wr2_d"]
    b2t_d = env["b2t_d"]; oneh_d = env["oneh_d"]
    wms_d = env["wms_d"]; bms_d = env["bms_d"]; mu_d = env["mu_d"]
    sg_d = env["sg_d"]; dbg_d = env["dbg_d"]
    MT = R // 128
    TSL = 4  # timesteps per head slice (small slices avoid a PE spike)

    with tile.TileContext(nc) as tc, ExitStack() as top:  # noqa: SIM117
        singles = top.enter_context(tc.tile_pool(name="singles", bufs=1))
        # scan pools are opened BEFORE the phase-1 pools so they never share
        # PSUM banks / SBUF ranges with them (sharing would add WAR waits
        # serializing scan start behind the last phase-1 gather/transpose)
        ew = top.enter_context(tc.tile_pool(name="ew", bufs=8))
        pg1p = top.enter_context(tc.tile_pool(name="pg1", bufs=1, space="PSUM"))
        pg2p = top.enter_context(tc.tile_pool(name="pg2", bufs=1, space="PSUM"))
        psh = top.enter_context(tc.tile_pool(name="psh", bufs=1, space="PSUM"))
        hew = top.enter_context(tc.tile_pool(name="hew", bufs=2))

        # ---------------- constants / weights to SBUF ----------------
        # small input tensors first so they don't queue behind 6.3MB of
        # weights on the DMA ring (phase 1 needs them immediately)
        singles_idx = singles
        idx0_sb = singles_idx.tile([128, MT], I32)
        nc.sync.dma_start(out=idx0_sb[:], in_=idx0_d[:])
        xcb = singles.tile([128, MT, F], F32)
        nc.sync.dma_start(out=xcb[:], in_=xcr_d[:])
        iota1_sb = singles.tile([CARD1, 1], F32)
        nc.sync.dma_start(out=iota1_sb[:], in_=iota1_d[:])
        e1t_sb = singles.tile([CARD1, E1], BF16)
        nc.sync.dma_start(out=e1t_sb[:], in_=e1t_d[:])
        idx1rep = singles.tile([CARD1, R], BF16)
        idx1_rep_src = bass.AP(tensor=idx1r_d[:].tensor, offset=0,
                               ap=[[0, CARD1], [1, R]])
        nc.sync.dma_start(out=idx1rep[:], in_=idx1_rep_src)
        w1x_sb = singles.tile([64, G4], BF16)
        nc.sync.dma_start(out=w1x_sb[:], in_=w1x_d[:])
        # big weight loads in per-chunk pieces so the short per-tile gather
        # transfers can interleave on the (serial) DMA engine
        wr1_sb = singles.tile([128, KC, G4], BF16)
        for c in range(KC):
            nc.sync.dma_start(out=wr1_sb[:, c, :], in_=wr1_d[:, c, :])
        b2t_sb = singles.tile([16, 128], BF16)
        nc.sync.dma_start(out=b2t_sb[:], in_=b2t_d[:])
        oneh_sb = singles.tile([16, 512], BF16)
        nc.sync.dma_start(out=oneh_sb[:], in_=oneh_d[:])
        wms_sb = singles.tile([128, KC, 2], BF16)
        nc.sync.dma_start(out=wms_sb[:], in_=wms_d[:])
        bms_sb = singles.tile([1, 2], F32)
        nc.sync.dma_start(out=bms_sb[:], in_=bms_d[:])
        wk2_sb = singles.tile([128, KC, G4], BF16)
        for c in range(KC):
            nc.sync.dma_start(out=wk2_sb[:, c, :], in_=wk2_d[:, c, :])
        wr2_sb = singles.tile([128, KC, G4], BF16)
        for c in range(KC):
            nc.sync.dma_start(out=wr2_sb[:, c, :], in_=wr2_d[:, c, :])

        ident_f32 = singles.tile([128, 128], F32)
        make_identity(nc, ident_f32[:])

        # x^T as one tile PER 128-column block: dependencies are tracked at
        # tile granularity, so a single xT tile would make scan step 0 wait
        # for the LAST phase-1 gather (~100us of serial indirect DMAs)
        xTs = [singles.tile([64, 128], BF16, name=f"xT{m}", tag=f"xT{m}")
               for m in range(MT)]
        h1T = singles.tile([128, 4, KC, BSH], BF16)   # 4-deep (L2 lags 2 steps)
        h2T = singles.tile([128, KC, T, BSH], BF16)   # full history (head)
        c1 = singles.tile([128, 128], BF16)
        c2 = singles.tile([128, 128], BF16)
        sgacc = singles.tile([1, T * BSH], F32)   # raw sigma pre-activations
        exp_all = singles.tile([1, T * BSH], BF16)
        nc.vector.memset(c1[:], 0.0)
        nc.vector.memset(c2[:], 0.0)

        # ---------------- phase 1: build x^T ----------------
        # The per-tile work is emitted INTERLEAVED into the scan loop with an
        # 8-tile lookahead: the in-order PE queue otherwise places each
        # transpose far ahead of its gather's completion and every early scan
        # step stalls on the (1us-per-gather) software-DGE pipeline.
        gp = top.enter_context(tc.tile_pool(name="gather", bufs=1))
        ptr = top.enter_context(tc.tile_pool(name="gtr", bufs=1, space="PSUM"))
        # e1 lookup as a one-hot matmul (CARD1=100 <= 128): cheaper than
        # 48 more 1us software-DGE gathers on the Pool engine
        onehot1 = gp.tile([CARD1, R], BF16)
        nc.vector.tensor_scalar(onehot1[:], idx1rep[:], iota1_sb[:],
                                None, op0=mybir.AluOpType.is_equal)

        def emit_tile(m):
            # assembled rows: [p, 64] = [e0 | e1(pad) | xc | ones]
            # NOTE: multi-index indirect DMA is broken on HW (stomps memory);
            # one gather per 128-row tile, single idx column each.
            asm = gp.tile([128, 64], F32, name=f"asm{m}", tag=f"asm{m}")
            nc.vector.memset(asm[:], 1.0)
            nc.gpsimd.tensor_copy(asm[:, E0 + E1:DIN], xcb[:, m, :])
            nc.gpsimd.indirect_dma_start(
                out=asm[:, 0:E0], out_offset=None, in_=e0t_d[:],
                in_offset=bass.IndirectOffsetOnAxis(
                    ap=idx0_sb[:, m:m + 1], axis=0))
            ps = ptr.tile([80, 128], F32, name=f"ps{m}", tag="ps")
            nc.tensor.transpose(ps[0:64, :], asm[:], ident_f32[:])
            nc.vector.tensor_copy(xTs[m][:], ps[0:64, :])
            nc.tensor.matmul(ps[64:80, :], e1t_sb[:],
                             onehot1[:, 128 * m:128 * (m + 1)],
                             start=True, stop=True)
            nc.vector.tensor_copy(xTs[m][E0:E0 + E1, :], ps[64:80, :])

        PRO = 4   # tiles emitted before the scan starts (supply lookahead)

        if upto == "xT":
            for m in range(MT):
                emit_tile(m)
            with tc.tile_pool(name="dbgp", bufs=1) as dp:
                dbg_sb = dp.tile([64, R], F32)
                for m in range(MT):
                    nc.vector.tensor_copy(
                        dbg_sb[:, 128 * m:128 * (m + 1)], xTs[m][:])
                nc.sync.dma_start(out=dbg_d[:], in_=dbg_sb[:])
            return
        for m in range(PRO):
            emit_tile(m)

        # -------- phase 2: fused transposed-gates scan (L2 two steps behind,
        # so every matmul of a block is ready when the block's run starts and
        # L2's psum accumulation group never blocks L1 on the in-order PE
        # queue) --------
        if True:
            # gates psum is split into three tiles per step — (i|f), (o), (g) —
            # because RAW deps are tile-granular: one [128,512] tile would make
            # every gate activation wait for ALL 80 matmuls of the step.
            def gate_slot(pool, tagp, merged):
                if merged:   # (i|f|o) in one tile, g separate
                    return (pool.tile([128, 384], F32, name=tagp + "ifo",
                                      tag=tagp + "ifo"),
                            None,
                            pool.tile([128, 128], F32, name=tagp + "g",
                                      tag=tagp + "g"))
                return (pool.tile([128, 256], F32, name=tagp + "if", tag=tagp + "if"),
                        pool.tile([128, 128], F32, name=tagp + "o", tag=tagp + "o"),
                        pool.tile([128, 128], F32, name=tagp + "g", tag=tagp + "g"))

            def gate_out(pgs, m):
                pg_if, pg_o, pg_g = pgs
                if m >= 12:
                    return pg_g[:, 32 * (m - 12):32 * (m - 12) + 32]
                if pg_o is None:
                    return pg_if[:, 32 * m:32 * m + 32]
                if m < 8:
                    return pg_if[:, 32 * m:32 * m + 32]
                return pg_o[:, 32 * (m - 8):32 * (m - 8) + 32]

            def gate_tail(pgs, c_st, hview, eng):
                """sig(i|f[|o]), tanh(g), c' = f*c + i*g, h = o*tanh(c')."""
                pg_if, pg_o, pg_g = pgs
                tg = ew.tile([128, 128], BF16, tag="tg")
                nc.scalar.activation(tg[:], pg_g[:], A.Tanh)
                nif = 384 if pg_o is None else 256
                sif = ew.tile([128, nif], BF16, tag="sif")
                nc.scalar.activation(sif[:], pg_if[:], A.Sigmoid)
                ig = ew.tile([128, 128], BF16, tag="ig")
                eng.tensor_mul(ig[:], sif[:, 0:128], tg[:])
                fc = ew.tile([128, 128], BF16, tag="fc")
                eng.tensor_mul(fc[:], sif[:, 128:256], c_st[:])
                eng.tensor_add(c_st[:], fc[:], ig[:])
                if pg_o is None:
                    so = sif[:, 256:384]
                else:
                    so_t = ew.tile([128, 128], BF16, tag="so")
                    nc.scalar.activation(so_t[:], pg_o[:], A.Sigmoid)
                    so = so_t[:]
                tc_ = ew.tile([128, 128], BF16, tag="tc")
                nc.scalar.activation(tc_[:], c_st[:], A.Tanh)
                eng.tensor_mul(hview, so, tc_[:])

            def l1_step(t):
                pgs = gate_slot(pg1p, "p1", False)
                xsrc = xTs[t // 4]
                xsl = slice((t % 4) * BSH, (t % 4 + 1) * BSH)
                for m in _M_ORDER:
                    osl = gate_out(pgs, m)
                    nc.tensor.matmul(osl, w1x_sb[0:DIN + 1, 128 * m:128 * (m + 1)],
                                     xsrc[0:DIN + 1, xsl],
                                     start=True, stop=(t == 0))
                    if t > 0:
                        for c in range(KC):
                            nc.tensor.matmul(
                                osl, wr1_sb[:, c, 128 * m:128 * (m + 1)],
                                h1T[:, (t - 1) % 4, c, :],
                                start=False, stop=(c == KC - 1))
                hv = h1T[:, t % 4, :, :]
                gate_tail(pgs, c1, hv, nc.vector)

            def l2_step(s):
                pgs = gate_slot(pg2p, "p2", False)
                # b2 seed: one K=16 one-hot matmul per gate tile
                nc.tensor.matmul(pgs[0][:], b2t_sb[:], oneh_sb[:, 0:256],
                                 start=True, stop=False, skip_group_check=True)
                nc.tensor.matmul(pgs[1][:], b2t_sb[:], oneh_sb[:, 256:384],
                                 start=True, stop=False, skip_group_check=True)
                nc.tensor.matmul(pgs[2][:], b2t_sb[:], oneh_sb[:, 384:512],
                                 start=True, stop=False, skip_group_check=True)
                for m in _M_ORDER:
                    osl = gate_out(pgs, m)
                    if s > 0:
                        for c in range(KC):
                            nc.tensor.matmul(
                                osl, wr2_sb[:, c, 128 * m:128 * (m + 1)],
                                h2T[:, c, s - 1, :],
                                start=False, stop=False, skip_group_check=True)
                    for c in range(KC):
                        nc.tensor.matmul(
                            osl, wk2_sb[:, c, 128 * m:128 * (m + 1)],
                            h1T[:, s % 4, c, :],
                            start=False, stop=(c == KC - 1),
                            skip_group_check=True)
                hv = bass.AP(tensor=h2T.tensor, offset=h2T.offset + s * BSH,
                             ap=[list(h2T.ap[0]), [T * BSH, KC], [1, BSH]])
                gate_tail(pgs, c2, hv, nc.vector)

            def head_slice(n):
                ps_head = psh.tile([33, TSL * BSH], F32, tag="hm")
                ps_mu = ps_head[0:1, :]
                ps_sg = ps_head[32:33, :]
                for c in range(KC):
                    rhs = h2T[:, c, n * TSL:(n + 1) * TSL, :]
                    nc.tensor.matmul(ps_mu, wms_sb[:, c, 0:1], rhs,
                                     start=(c == 0), stop=(c == KC - 1))
                    nc.tensor.matmul(ps_sg, wms_sb[:, c, 1:2], rhs,
                                     start=(c == 0), stop=(c == KC - 1))
                mu_sl = hew.tile([1, TSL * BSH], F32)
                nc.vector.tensor_scalar_add(mu_sl[:], ps_mu, bms_sb[0:1, 0:1])
                nc.vector.tensor_copy(
                    sgacc[0:1, TSL * BSH * n:TSL * BSH * (n + 1)], ps_sg)
                mu_view = bass.AP(tensor=mu_d[:].tensor, offset=n * TSL,
                                  ap=[[0, 1], [1, TSL], [T, BSH]])
                nc.sync.dma_start(out=mu_view, in_=mu_sl[:])

            for t in range(T + 2):
                if t % 4 == 0 and PRO + t // 4 < MT:
                    emit_tile(PRO + t // 4)
                if t < T:
                    l1_step(t)
                if t >= 2:
                    s = t - 2
                    l2_step(s)
                    if (s + 1) % TSL == 0:
                        head_slice((s + 1) // TSL - 1)

            # sigma = softplus(raw + bsig), batched once: Exp/Ln tables are in
            # a different act-func set than Sigmoid/Tanh, so doing this inside
            # the scan would cost two 1.3us table swaps every 16 steps
            nc.scalar.activation(exp_all[:], sgacc[:], A.Exp,
                                 bias=bms_sb[0:1, 1:2])
            nc.scalar.activation(sgacc[:], exp_all[:], A.Ln, bias=1.0)
            sg_view = bass.AP(
                tensor=sg_d[:].tensor, offset=0,
                ap=[[0, 1], [TSL, T // TSL], [1, TSL], [T, BSH]])
            nc.sync.dma_start(out=sg_view, in_=sgacc[:])

    return nc


def _marshal(inputs):
    """Host-side shard/layout marshalling (no compute beyond dtype cast/pad)."""
    bf = ml_dtypes.bfloat16
    xc = np.ascontiguousarray(np.asarray(inputs["x_cont"], np.float32))
    cat0 = np.asarray(inputs["cat0"]).astype(np.int32)
    cat1 = np.asarray(inputs["cat1"]).astype(np.int32)
    emb0 = np.asarray(inputs["emb0"], np.float32)
    emb1 = np.asarray(inputs["emb1"], np.float32)
    Wk1 = np.asarray(inputs["Wk1"], np.float32)
    Wr1 = np.asarray(inputs["Wr1"], np.float32)
    b1 = np.asarray(inputs["b1"], np.float32)
    Wk2 = np.asarray(inputs["Wk2"], np.float32)
    Wr2 = np.asarray(inputs["Wr2"], np.float32)
    b2 = np.asarray(inputs["b2"], np.float32)
    Wmu = np.asarray(inputs["Wmu"], np.float32)
    bmu = np.asarray(inputs["bmu"], np.float32)
    Wsig = np.asarray(inputs["Wsig"], np.float32)
    bsig = np.asarray(inputs["bsig"], np.float32)

    # permute gate columns [i, f, g, o] -> [i, f, o, g] so sigmoid gates are
    # contiguous in the transposed-gates free layout
    def perm(W):
        return np.concatenate(
            [W[..., 0:H], W[..., H:2 * H], W[..., 3 * H:4 * H],
             W[..., 2 * H:3 * H]], axis=-1)

    Wk1p, Wr1p, b1p = perm(Wk1), perm(Wr1), perm(b1)
    Wk2p, Wr2p, b2p = perm(Wk2), perm(Wr2), perm(b2)

    # x^T partition order: 0-31 emb0 dims, 32-47 emb1 dims, 48-55 x_cont, 56 ones
    w1x = np.zeros((64, G4), bf)
    w1x[0:E0, :] = Wk1p[F:F + E0, :].astype(bf)
    w1x[E0:E0 + E1, :] = Wk1p[F + E0:DIN, :].astype(bf)
    w1x[E0 + E1:DIN, :] = Wk1p[0:F, :].astype(bf)
    w1x[DIN, :] = b1p.astype(bf)

    wr1 = np.zeros((128, KC, G4), bf)
    wk2 = np.zeros((128, KC, G4), bf)
    wr2 = np.zeros((128, KC, G4), bf)
    wms = np.zeros((128, KC, 2), bf)
    for c in range(KC):
        wr1[:, c, :] = Wr1p[c * 128:(c + 1) * 128, :].astype(bf)
        wk2[:, c, :] = Wk2p[c * 128:(c + 1) * 128, :].astype(bf)
        wr2[:, c, :] = Wr2p[c * 128:(c + 1) * 128, :].astype(bf)
        wms[:, c, 0] = Wmu[c * 128:(c + 1) * 128, 0].astype(bf)
        wms[:, c, 1] = Wsig[c * 128:(c + 1) * 128, 0].astype(bf)
    b2t = np.ascontiguousarray(b2p.reshape(16, 128).astype(bf))
    oneh = np.kron(np.eye(16, dtype=np.float32),
                   np.ones((1, BSH), np.float32)).astype(bf)
    bms = np.array([[float(bmu.reshape(-1)[0]), float(bsig.reshape(-1)[0])]],
                   np.float32)

    MT = R // 128

    def wrap_idx(cat):  # [BSH, T] -> (t,b) rows -> [128, MT] int32
        lin = np.ascontiguousarray(cat.T).reshape(-1)       # (t, b) order
        return np.ascontiguousarray(lin.reshape(MT, 128).T.astype(np.int32))

    iota1 = np.arange(CARD1, dtype=np.float32).reshape(CARD1, 1)
    e1t16 = emb1.astype(bf)

    in_maps = []
    for cidx in range(NC_N):
        sl = slice(cidx * BSH, (cidx + 1) * BSH)
        xcs = xc[sl]                                        # [32, 192, 8]
        rows = xcs.transpose(1, 0, 2).reshape(R, F)      # (t,b) rows
        xcr = np.ascontiguousarray(
            rows.reshape(MT, 128, F).transpose(1, 0, 2).astype(np.float32))
        idx1r = np.ascontiguousarray(
            cat1[sl].T.reshape(1, R).astype(np.float32)).astype(bf)
        in_maps.append({
            "xcr": xcr,
            "idx0": wrap_idx(cat0[sl]),
            "idx1r": idx1r, "iota1": iota1,
            "e0tab": emb0, "e1t16": e1t16,
            "w1x": w1x, "wr1": wr1, "wk2": wk2, "wr2": wr2,
            "b2t": b2t, "oneh": oneh,
            "wms": wms, "bms": bms,
        })
    return in_maps


_RUN_KWARGS = {}   # test harness may set e.g. {"trace": True} for profiling
_LAST_RESULT = []


def kernel(**inputs):
    from concourse.bass_utils import run_bass_kernel_spmd
    in_maps = _marshal(inputs)
    nc = build_nc()
    res = run_bass_kernel_spmd(nc, in_maps, core_ids=list(range(NC_N)),
                               **_RUN_KWARGS)
    _LAST_RESULT.clear()
    _LAST_RESULT.append(res)
    mu = np.concatenate([r["mu"] for r in res.results], axis=0)      # [256, 192]
    sg = np.concatenate([r["sigma"] for r in res.results], axis=0)
    return (mu.reshape(B, T, 1).astype(np.float32),
            sg.reshape(B, T, 1).astype(np.float32))
